# revision 4
# baseline (speedup 1.0000x reference)
"""FM layer (first + second order factorization machine) on 8 TRN2 NeuronCores.

Batch-parallel: the full embedding table (w | V^T rows, [2_600_013, 17] f32)
is replicated to every core's HBM; each core handles 512 of the 4096 batch
rows and gathers its 512*26 rows of 68B via per-column indirect DMAs
(the SWDGE ucode reads exactly one index per partition per instruction, so
104 instructions per core — this ~1.4us/instruction Pool-engine stream is
the dominant cost and sits at the hardware floor for this primitive).

Raw bass (no TileContext): hand-scheduled semaphores cut the Tile preamble,
scheduling gaps and teardown (~4us total vs the Tile version).

Math per batch row b (fields f=1..26, latent k=1..16):
  idx[b,f] = sparse[b,f] + 13 + f*100000
  first  = w0 + d@wd + sum_f w[idx]
  e      = d@Vd + sum_f Vt[idx]          (16-vec)
  sq     = d^2@Vd^2 + sum_f Vt[idx]^2    (scalar after sum_k)
  out    = first + 0.5*(sum_k e^2 - sq)
Dense part via one PE matmul per 128-row tile:
  lhsT = [d^T; (d^2)^T; ones] (27 x 128), rhs (27 x 18):
  mm[:, 0] = w0 + d@wd, mm[:, 1:17] = d@Vd, mm[:, 17] = sum_k (d^2@Vd^2)[k].
"""

import sys

sys.path.insert(0, "/opt/trn_rl_repo")

from contextlib import ExitStack

import numpy as np

import concourse.bass as bass
import concourse.mybir as mybir

N_DENSE = 13
N_FIELDS = 26
PER_FIELD = 100000
FEATURE_NUM = N_FIELDS * PER_FIELD + N_DENSE  # 2_600_013
K = 16
BATCH = 4096
N_CORES = 8
BL = BATCH // N_CORES  # 512
P = 128
T = BL // P  # 4 tiles of 128 rows
ROW = 1 + K  # 17 floats per table row
KM = 2 * N_DENSE + 1  # 27
NO = ROW + 1  # 18
COLS = T * N_FIELDS  # 104 gather columns per core
GW = 18  # gather column width in SBUF (17 used + 1 pad so the AP stays 3-dim)

SPLITS = [2, 24, 26, 26, 26]  # tiny first chunk so gather 0 starts sooner
PERCOL = True  # one indirect DMA per column (HW supports only idx[p,0])

F32 = mybir.dt.float32
I32 = mybir.dt.int32


def build_nc() -> bass.Bass:
    assert sum(SPLITS) == COLS
    bounds = []
    c = 0
    for w in SPLITS:
        bounds.append((c, c + w))
        c += w

    nc = bass.Bass()
    table = nc.dram_tensor("table", [FEATURE_NUM, ROW], F32, kind="ExternalInput")
    idx = nc.dram_tensor("idx", [P, COLS], I32, kind="ExternalInput")
    dmat = nc.dram_tensor("dmat", [KM, BL + NO], F32, kind="ExternalInput")
    out = nc.dram_tensor("out", [P, T], F32, kind="ExternalOutput")

    sI = [nc.alloc_semaphore(f"sI{i}") for i in range(len(SPLITS))]
    sG = [nc.alloc_semaphore(f"sG{i}") for i in range(len(SPLITS))]
    sD = nc.alloc_semaphore("sD")
    sM = nc.alloc_semaphore("sM")
    sV = nc.alloc_semaphore("sV")
    sA = nc.alloc_semaphore("sA")
    sO = nc.alloc_semaphore("sO")

    ctx = ExitStack()
    idx_t = ctx.enter_context(nc.sbuf_tensor("idx_t", [P, COLS], I32))
    dmat_t = ctx.enter_context(nc.sbuf_tensor("dmat_t", [KM, BL + NO], F32))
    g = ctx.enter_context(nc.sbuf_tensor("g", [P, COLS * GW], F32))
    sf_all = ctx.enter_context(nc.sbuf_tensor("sf_all", [P, T * ROW], F32))
    sqs = ctx.enter_context(nc.sbuf_tensor("sqs", [P, T * N_FIELDS * K], F32))
    s2_all = ctx.enter_context(nc.sbuf_tensor("s2_all", [P, T], F32))
    ts_all = ctx.enter_context(nc.sbuf_tensor("ts_all", [P, T * ROW], F32))
    sq_all = ctx.enter_context(nc.sbuf_tensor("sq_all", [P, T * K], F32))
    se2_all = ctx.enter_context(nc.sbuf_tensor("se2_all", [P, T], F32))
    d1 = ctx.enter_context(nc.sbuf_tensor("d1", [P, T], F32))
    d2 = ctx.enter_context(nc.sbuf_tensor("d2", [P, T], F32))
    hf = ctx.enter_context(nc.sbuf_tensor("hf", [P, T], F32))
    out_t = ctx.enter_context(nc.sbuf_tensor("out_t", [P, T], F32))
    mm = nc.alloc_psum_tensor("mm", [P, T * NO], F32)

    # --- uploads ---
    # idx chunks on the SP (sync) HWDGE ring; dmat on the ACT ring in parallel
    for i, (c0, c1) in enumerate(bounds):
        nc.sync.dma_start(idx_t[:, c0:c1], idx[:, c0:c1]).then_inc(sI[i], 16)
    nc.scalar.dma_start(dmat_t[:], dmat[:]).then_inc(sD, 16)

    # --- dense matmuls (PE) ---
    import os as _os
    if _os.environ.get("K2_BISECT", "") == "1":
        # bisect mode: skip PE/PSUM entirely; mm-equivalent zeros in SBUF
        mm = ctx.enter_context(nc.sbuf_tensor("mmz", [P, T * NO], F32))
        nc.scalar.wait_ge(sD, 16)  # keep the dep shape
        for t in range(T):
            nc.vector.memset(mm[:, t * NO : (t + 1) * NO], 0.0).then_inc(sM, 1)
    else:
        nc.tensor.wait_ge(sD, 16)
        for t in range(T):
            nc.tensor.matmul(
                mm[:, t * NO : (t + 1) * NO],
                dmat_t[:, t * P : (t + 1) * P],
                dmat_t[:, BL : BL + NO],
                start=True,
                stop=True,
            ).then_inc(sM, 1)

    # --- gathers (Pool SWDGE) ---
    if PERCOL:
        nc.gpsimd.wait_ge(sI[0], 16)
        for i, (c0, c1) in enumerate(bounds):
            if i > 0:
                nc.gpsimd.wait_ge(sI[i], 16)
            for col in range(c0, c1):
                nc.gpsimd.indirect_dma_start(
                    out=g[:, col * GW : col * GW + ROW],
                    out_offset=None,
                    in_=table[:],
                    in_offset=bass.IndirectOffsetOnAxis(
                        ap=idx_t[:, col : col + 1], axis=0
                    ),
                ).then_inc(sG[i], 16)
    else:
        for i, (c0, c1) in enumerate(bounds):
            nc.gpsimd.wait_ge(sI[i], 16)
            nc.gpsimd.indirect_dma_start(
                out=g[:, c0 * GW : c1 * GW].rearrange(
                    "p (n c) -> p n c", n=c1 - c0
                )[:, :, 0:ROW],
                out_offset=None,
                in_=table[:],
                in_offset=bass.IndirectOffsetOnAxis(ap=idx_t[:, c0:c1], axis=0),
            ).then_inc(sG[i], 16)

    # --- per-tile reductions ---
    # DVE: sf_all[:, t*17:(t+1)*17] = sum_f g[:, (t*26+f)*17 + c]
    # ACT: s2_all[:, t] = sum_{f,k} g[:, (t*26+f)*17 + 1+k]^2
    # Every DVE op bumps sV, every ACT compute op bumps sA (counting sems);
    # cross-dependency waits are standalone wait_ge instructions.
    chunk_of_tile = []
    for t in range(T):
        last_col = (t + 1) * N_FIELDS - 1
        for i, (c0, c1) in enumerate(bounds):
            if c0 <= last_col < c1:
                chunk_of_tile.append(i)
                break
    dve_waited = [False] * len(SPLITS)
    act_waited = [False] * len(SPLITS)
    def _wait_upto(eng, waited, j):
        for k in range(j + 1):
            if not waited[k]:
                tgt = 16 * (bounds[k][1] - bounds[k][0]) if PERCOL else 16
                eng.wait_ge(sG[k], tgt)
                waited[k] = True
    for t in range(T):
        i = chunk_of_tile[t]
        _wait_upto(nc.vector, dve_waited, i)
        gt = g[:, t * N_FIELDS * GW : (t + 1) * N_FIELDS * GW]
        nc.vector.tensor_reduce(
            out=sf_all[:, t * ROW : (t + 1) * ROW],
            in_=gt.rearrange("p (f c) -> p c f", f=N_FIELDS)[:, 0:ROW, :],
            axis=mybir.AxisListType.X,
            op=mybir.AluOpType.add,
        ).then_inc(sV, 1)
    for t in range(T):
        i = chunk_of_tile[t]
        _wait_upto(nc.scalar, act_waited, i)
        gt = g[:, t * N_FIELDS * GW : (t + 1) * N_FIELDS * GW]
        nc.scalar.activation(
            out=sqs[:, t * N_FIELDS * K : (t + 1) * N_FIELDS * K].rearrange(
                "p (f c) -> p f c", f=N_FIELDS),
            in_=gt.rearrange("p (f c) -> p f c", f=N_FIELDS)[:, :, 1:ROW],
            func=mybir.ActivationFunctionType.Square,
            accum_out=s2_all[:, t : t + 1],
        ).then_inc(sA, 1)

    # --- combine (batched over the 4 tiles) ---
    mm_tc = mm[:].rearrange("p (t c) -> p t c", t=T)
    ts_tc = ts_all[:].rearrange("p (t c) -> p t c", t=T)

    nc.vector.wait_ge(sM, 4)
    nc.vector.wait_ge(sV, 4)
    nc.vector.tensor_tensor(          # sV=5
        out=ts_tc,
        in0=sf_all[:].rearrange("p (t c) -> p t c", t=T),
        in1=mm_tc[:, :, 0:ROW],
        op=mybir.AluOpType.add,
    ).then_inc(sV, 1)

    nc.scalar.wait_ge(sV, 5)
    nc.scalar.activation(             # sA=5
        out=sq_all[:].rearrange("p (t c) -> p t c", t=T),
        in_=ts_tc[:, :, 1:ROW],
        func=mybir.ActivationFunctionType.Square,
    ).then_inc(sA, 1)

    nc.vector.wait_ge(sA, 5)
    nc.vector.tensor_reduce(          # sV=6
        out=se2_all[:],
        in_=sq_all[:].rearrange("p (t c) -> p t c", t=T),
        axis=mybir.AxisListType.X,
        op=mybir.AluOpType.add,
    ).then_inc(sV, 1)
    nc.vector.wait_ge(sV, 6)
    nc.vector.tensor_tensor(          # sV=7
        out=d1[:], in0=se2_all[:], in1=s2_all[:], op=mybir.AluOpType.subtract
    ).then_inc(sV, 1)
    nc.vector.wait_ge(sV, 7)
    nc.vector.tensor_tensor(          # sV=8
        out=d2[:].rearrange("p (t o) -> p t o", t=T),
        in0=d1[:].rearrange("p (t o) -> p t o", t=T),
        in1=mm_tc[:, :, ROW : ROW + 1],
        op=mybir.AluOpType.subtract,
    ).then_inc(sV, 1)
    nc.vector.wait_ge(sV, 8)
    nc.vector.tensor_scalar_mul(out=hf[:], in0=d2[:], scalar1=0.5).then_inc(sV, 1)
    nc.vector.wait_ge(sV, 9)
    nc.vector.tensor_tensor(          # sV=10
        out=out_t[:].rearrange("p (t o) -> p t o", t=T),
        in0=hf[:].rearrange("p (t o) -> p t o", t=T),
        in1=ts_tc[:, :, 0:1],
        op=mybir.AluOpType.add,
    ).then_inc(sV, 1)

    # --- store ---
    nc.sync.wait_ge(sV, 10)
    nc.sync.dma_start(out[:], out_t[:]).then_inc(sO, 16)
    nc.sync.wait_ge(sO, 16)

    mybir.codegen_inst_isa_subclasses(nc)
    return nc


def prepare_inputs(dense_inputs, sparse_inputs, w0, w, V):
    dense = np.asarray(dense_inputs, dtype=np.float32)
    sparse = np.asarray(sparse_inputs, dtype=np.int32)
    w0 = np.asarray(w0, dtype=np.float32).reshape(-1)
    w = np.asarray(w, dtype=np.float32).reshape(FEATURE_NUM, 1)
    V = np.asarray(V, dtype=np.float32)

    table = np.ascontiguousarray(
        np.concatenate([w, V.T], axis=1), dtype=np.float32
    )

    offsets = (N_DENSE + np.arange(N_FIELDS, dtype=np.int32) * PER_FIELD).astype(
        np.int32
    )
    gidx = sparse + offsets[None, :]  # [B, 26]

    wd = w[:N_DENSE, 0]
    Vd = V[:, :N_DENSE].T.astype(np.float32)  # [13, 16]
    u = (Vd * Vd).sum(axis=1)

    rhs = np.zeros((KM, NO), dtype=np.float32)
    rhs[:N_DENSE, 0] = wd
    rhs[:N_DENSE, 1:ROW] = Vd
    rhs[N_DENSE : 2 * N_DENSE, ROW] = u
    rhs[2 * N_DENSE, 0] = w0[0]

    in_maps = []
    for c in range(N_CORES):
        dslice = dense[c * BL : (c + 1) * BL]
        dmat = np.empty((KM, BL + NO), dtype=np.float32)
        dmat[:N_DENSE, :BL] = dslice.T
        dmat[N_DENSE : 2 * N_DENSE, :BL] = (dslice * dslice).T
        dmat[2 * N_DENSE, :BL] = 1.0
        dmat[:, BL:] = rhs

        gslice = gidx[c * BL : (c + 1) * BL]  # [512, 26]
        idx_arr = np.ascontiguousarray(
            gslice.reshape(T, P, N_FIELDS).transpose(1, 0, 2).reshape(P, COLS)
        ).astype(np.int32)

        in_maps.append({"table": table, "idx": idx_arr, "dmat": dmat})
    return in_maps


def assemble_output(results):
    out = np.empty((BATCH, 1), dtype=np.float32)
    for c in range(N_CORES):
        o = results[c]["out"]  # [128, T]
        out[c * BL : (c + 1) * BL, 0] = o.T.reshape(BL)
    return out


_NC_CACHE = None


def kernel(**inputs) -> np.ndarray:
    global _NC_CACHE
    from concourse.bass_utils import run_bass_kernel_spmd

    if _NC_CACHE is None:
        _NC_CACHE = build_nc()
    nc = _NC_CACHE
    in_maps = prepare_inputs(**inputs)
    last_err = None
    for _ in range(3):
        try:
            res = run_bass_kernel_spmd(nc, in_maps, list(range(N_CORES)))
            return assemble_output(res.results)
        except Exception as e:  # noqa: BLE001
            last_err = e
    raise last_err
